# revision 1
# baseline (speedup 1.0000x reference)
"""Trainium2 Bass kernel for nn_CascadeGNN (2-layer GCN + mean/max pool + cls).

Strategy (8 NeuronCores, data-parallel over graphs):
  - Nodes/edges sharded by graph id (batch is sorted -> contiguous shards,
    16 graphs per core). Each graph gets a fixed slot of TG node tiles so the
    SPMD program is uniform across cores. Edges live on the core owning dst.
  - Key identity: with u = dis * h, a GCN layer is
        h' = relu(dis * (sum_{e: src->n} u[src] + u[n]) @ W + b)
    (row scaling and the right weight multiply commute with the segment sum),
    so cores exchange only the small u tables and apply W post-aggregation.
  - Per 128-node tile, edge messages are gathered with dma_gather (bulk SWDGE
    gather, int16 indices -> the padded table is split in <=32767-row
    quarters) and reduced on the TensorEngine via one-hot matrices
    M[e, n] = (dst_local[e] == n) built on the VectorEngine (iota+is_equal).
    PSUM accumulates the segment sum; the self term is an identity matmul
    from a core-local copy of the table shard.
  - u tables are stored in a "primed" partition-major row order
    (row' = p*T + c for node tile c, partition p) so table writes are large
    fully-contiguous DMAs; gather indices are relabeled on the host.
  - u0 is computed replicated (dense embedding is cheap); u1 is exchanged
    with one AllGather.
  - Pooling: segment mean via one-hot matmul; segment max via reduce_max over
    a transposed h2 slab (uniform per-graph spans); head + log_softmax
    on-device. Pad nodes are forced to zero (deg=1e38 -> dis~0, pad mask).

The Bass program is compiled per input instance (edge schedule baked in).
"""
import numpy as np

P = 128
NCORES = 8
H = 64
D_IN = 8
RUN = 4
PAD_DST = 20000.0
GPC = 16
PAD_DEG = 1.0e38

N = 100000
E = 1600000
G = 128
C = 2


# ----------------------------------------------------------------------------
# host-side metadata (sharding / index prep)
# ----------------------------------------------------------------------------

def build_meta(src, dst, batch):
    graph_start = np.searchsorted(batch, np.arange(G + 1))
    gsizes = (graph_start[1:] - graph_start[:-1]).astype(np.int64)
    TG = int(np.ceil(max(int(gsizes.max()), 1) / P))
    T = GPC * TG
    S_pad = T * P
    TBL = NCORES * S_pad
    NQ = int(np.ceil(TBL / 32767.0))
    QROWS = int(np.ceil(TBL / NQ / P)) * P

    # node -> padded table row (logical: local = tile*128 + partition)
    map_row = np.empty(N, np.int64)
    for g in range(G):
        k, slot = g // GPC, g % GPC
        a, b = graph_start[g], graph_start[g + 1]
        map_row[a:b] = k * S_pad + slot * TG * P + np.arange(b - a)

    deg = np.bincount(dst, minlength=N).astype(np.float64) + 1.0

    order = np.argsort(dst, kind="stable")
    src_s = src[order].astype(np.int64)
    dst_s = dst[order].astype(np.int64)
    # primed (partition-major) table row of the source
    sr = map_row[src_s]
    sk, sloc = sr // S_pad, sr % S_pad
    src_rowp = sk * S_pad + (sloc % P) * T + (sloc // P)
    src_q = src_rowp // QROWS
    src_rel = (src_rowp - src_q * QROWS).astype(np.int64)
    dst_row = map_row[dst_s]

    buckets = {}
    cnt = np.zeros((NCORES, T, NQ), np.int64)
    for k in range(NCORES):
        e0 = np.searchsorted(dst_row, k * S_pad)
        e1 = np.searchsorted(dst_row, (k + 1) * S_pad)
        loc = dst_row[e0:e1] - k * S_pad
        tq = loc // P
        t_start = e0 + np.searchsorted(tq, np.arange(T + 1))
        for t in range(T):
            a, b = t_start[t], t_start[t + 1]
            q_e = src_q[a:b]
            loc_t = loc[a - e0:b - e0] - t * P
            for q in range(NQ):
                m = q_e == q
                buckets[(k, t, q)] = (src_rel[a:b][m], loc_t[m])
                cnt[k, t, q] = int(m.sum())

    Gtq = (-(-cnt // P)).max(axis=0)

    n_runs = int(np.ceil(T / RUN))
    run_tiles = [list(range(r * RUN, min((r + 1) * RUN, T))) for r in range(n_runs)]
    runs = []
    col = 0
    sec_col = {}
    gcols = [[] for _ in range(T)]
    for tiles in run_tiles:
        run_col0 = col
        calls = []
        for q in range(NQ):
            ncols_q = int(sum(Gtq[t, q] for t in tiles))
            if ncols_q == 0:
                continue
            q_col0 = col
            for t in tiles:
                sec_col[(t, q)] = (col, int(Gtq[t, q]))
                gcols[t].extend(range(col, col + int(Gtq[t, q])))
                col += int(Gtq[t, q])
            calls.append(dict(q=q, col0=q_col0, ncols=ncols_q, NI=ncols_q * P))
        runs.append(dict(tiles=tiles, col0=run_col0, ncols=col - run_col0,
                         calls=calls))
    NCOL = col
    NSLOT = NCOL * P
    S = [len(g) for g in gcols]
    sched_of_tile = {}
    sc = 0
    for r in runs:
        for t in r["tiles"]:
            sched_of_tile[t] = sc
            sc += S[t]
    assert sc == NCOL

    per_core = []
    for k in range(NCORES):
        idx_lin = np.zeros(NSLOT, np.int16)
        slot_dl = np.full(NSLOT, -1, np.int64)
        for t in range(T):
            for q in range(NQ):
                if (t, q) not in sec_col:
                    continue
                c0, nc_ = sec_col[(t, q)]
                if nc_ == 0:
                    continue
                rel, dl = buckets[(k, t, q)]
                n = len(rel)
                off = c0 * P
                idx_lin[off:off + n] = rel.astype(np.int16)
                slot_dl[off:off + n] = dl
        idx16 = np.zeros((P, NSLOT // 16), np.int16)
        for r in runs:
            for cc in r["calls"]:
                c0, NI = cc["col0"], cc["NI"]
                lin = idx_lin[c0 * P: c0 * P + NI]
                w = lin.reshape(NI // 16, 16).T
                idx16[:, c0 * 8: c0 * 8 + NI // 16] = np.tile(w, (8, 1))
        dstl = np.full((P, NCOL), PAD_DST, np.float32)
        for t in range(T):
            sc0 = sched_of_tile[t]
            for j, c in enumerate(gcols[t]):
                sd = slot_dl[c * P:(c + 1) * P]
                dstl[:, sc0 + j] = np.where(sd >= 0, sd, PAD_DST).astype(np.float32)
        per_core.append(dict(idx16=idx16, dstl=dstl))

    def to_slot_layout(vals_per_node, pad_value, k):
        out = np.full(S_pad, pad_value, np.float32)
        for g in range(k * GPC, (k + 1) * GPC):
            a, b = graph_start[g], graph_start[g + 1]
            slot = g % GPC
            out[slot * TG * P: slot * TG * P + (b - a)] = vals_per_node[a:b]
        return out.reshape(T, P).T.copy()

    for k in range(NCORES):
        pc = per_core[k]
        pc["deg_own"] = to_slot_layout(deg.astype(np.float32), PAD_DEG, k)
        pc["batch_local"] = to_slot_layout(batch.astype(np.float32) - k * GPC,
                                           31.0, k)
        pc["padmask"] = to_slot_layout(np.ones(N, np.float32), 0.0, k)
        pc["cnt"] = np.maximum(gsizes[k * GPC:(k + 1) * GPC], 1).astype(
            np.float32).reshape(1, GPC)

    deg_all = np.full((P, NCORES * T), PAD_DEG, np.float32)
    for k in range(NCORES):
        deg_all[:, k * T:(k + 1) * T] = per_core[k]["deg_own"]

    return dict(
        T=T, TG=TG, S_pad=S_pad, TBL=TBL, NQ=NQ, QROWS=QROWS,
        NCOL=NCOL, NSLOT=NSLOT,
        runs=runs, gcols=gcols, S=S, sched_of_tile=sched_of_tile,
        graph_start=graph_start, map_row=map_row, deg_all=deg_all,
        gsizes=gsizes,
    ), per_core


def pack_xT4(x, meta, core=None):
    """x -> transposed layout [D_IN, n_tiles*P]: col tt*P+p = x[node(tt,p)]."""
    S_pad, TBL = meta["S_pad"], meta["TBL"]
    map_row = meta["map_row"]
    xp = np.zeros((TBL, D_IN), np.float32)
    xp[map_row] = x
    if core is not None:
        xp = xp[core * S_pad:(core + 1) * S_pad]
    return np.ascontiguousarray(xp.T), xp.shape[0] // P


# ----------------------------------------------------------------------------
# device program
# ----------------------------------------------------------------------------

def build_program(meta, stage=5, conv_parts="gmap"):
    import concourse.mybir as mybir
    import concourse.tile as tile
    from concourse import bacc
    from concourse.masks import make_identity

    f32 = mybir.dt.float32
    i16 = mybir.dt.int16
    i32 = mybir.dt.int32
    AF = mybir.ActivationFunctionType
    ALU = mybir.AluOpType
    AX = mybir.AxisListType

    T, TG, S_pad, TBL, NQ, QROWS, NCOL, NSLOT = (meta[k] for k in
        ["T", "TG", "S_pad", "TBL", "NQ", "QROWS", "NCOL", "NSLOT"])
    runs, gcols, S, sched_of_tile = (meta[k] for k in
        ["runs", "gcols", "S", "sched_of_tile"])
    SLAB = 16  # tiles per xT slab
    MAXS = max(max(S), 1)
    MAXRNC = max((r["ncols"] for r in runs), default=1)
    WB = 8  # tiles per prologue write batch (one PSUM bank: 8*64=512 f32)

    nc = bacc.Bacc("TRN2", target_bir_lowering=False)

    xT_all = nc.dram_tensor("xT_all", [D_IN, NCORES * S_pad], f32, kind="ExternalInput")
    xT_own = nc.dram_tensor("xT_own", [D_IN, S_pad], f32, kind="ExternalInput")
    deg_all_d = nc.dram_tensor("deg_all", [P, NCORES * T], f32, kind="ExternalInput")
    deg_own_d = nc.dram_tensor("deg_own", [P, T], f32, kind="ExternalInput")
    idx_d = nc.dram_tensor("idx16", [P, NSLOT // 16], i16, kind="ExternalInput")
    dstl_d = nc.dram_tensor("dstl", [P, NCOL], f32, kind="ExternalInput")
    batchl_d = nc.dram_tensor("batchl", [P, T], f32, kind="ExternalInput")
    padmask_d = nc.dram_tensor("padmask", [P, T], f32, kind="ExternalInput")
    cnt_d = nc.dram_tensor("cntg", [1, GPC], f32, kind="ExternalInput")
    W_emb_d = nc.dram_tensor("W_emb", [D_IN, H], f32, kind="ExternalInput")
    W_g1_d = nc.dram_tensor("W_g1", [H, H], f32, kind="ExternalInput")
    W_g2_d = nc.dram_tensor("W_g2", [H, H], f32, kind="ExternalInput")
    W_pool_d = nc.dram_tensor("W_pool", [2 * H, H], f32, kind="ExternalInput")
    W_cls_d = nc.dram_tensor("W_cls", [H, C], f32, kind="ExternalInput")
    b_emb_d = nc.dram_tensor("b_emb_r", [1, H], f32, kind="ExternalInput")
    b_g1_d = nc.dram_tensor("b_g1_r", [1, H], f32, kind="ExternalInput")
    b_g2_d = nc.dram_tensor("b_g2_r", [1, H], f32, kind="ExternalInput")
    b_pool_d = nc.dram_tensor("b_pool_c", [H, 1], f32, kind="ExternalInput")
    b_cls_d = nc.dram_tensor("b_cls_c", [C, 1], f32, kind="ExternalInput")
    out_d = nc.dram_tensor("out", [GPC, C], f32, kind="ExternalOutput")

    u0_tab = nc.dram_tensor("u0_tab", [TBL, H], f32)    # primed row order
    u0_own = nc.dram_tensor("u0_own", [S_pad, H], f32)  # primed row order
    u1_shard = nc.dram_tensor("u1_shard", [S_pad, H], f32)
    u1_tab = nc.dram_tensor("u1_tab", [TBL, H], f32)

    # primed views: [P, T*H] per core slab (partition p, tile-major contiguous)
    def primed_slab(tensor, k):
        return tensor[k * S_pad:(k + 1) * S_pad, :].rearrange(
            "(p c) f -> p (c f)", p=P)

    u0_tab_slabs = [primed_slab(u0_tab, k) for k in range(NCORES)]
    u0_own_p = primed_slab(u0_own, 0)
    u1_shard_p = primed_slab(u1_shard, 0)

    with tile.TileContext(nc) as tc:
        with (
            tc.tile_pool(name="const", bufs=1) as cp,
            tc.tile_pool(name="sbuf", bufs=2) as sp,
            tc.tile_pool(name="big", bufs=1) as bp,
            tc.tile_pool(name="psum", bufs=2, space="PSUM") as pp,
            tc.tile_pool(name="psum1", bufs=1, space="PSUM") as pp1,
        ):
            # ---------------- constants
            ident = cp.tile([P, P], f32)
            make_identity(nc, ident[:])
            iota_i = cp.tile([P, P], i32)
            nc.gpsimd.iota(iota_i[:], pattern=[[1, P]], base=0, channel_multiplier=0)
            iota_f = cp.tile([P, P], f32)
            nc.vector.tensor_copy(iota_f[:], iota_i[:])
            iota16_i = cp.tile([P, GPC], i32)
            nc.gpsimd.iota(iota16_i[:], pattern=[[1, GPC]], base=0, channel_multiplier=0)
            iota16_f = cp.tile([P, GPC], f32)
            nc.vector.tensor_copy(iota16_f[:], iota16_i[:])
            ones_row = cp.tile([1, P], f32)
            nc.gpsimd.memset(ones_row[:], 1.0)

            W_emb = cp.tile([D_IN, H], f32)
            nc.sync.dma_start(W_emb[:], W_emb_d[:])
            W_g1 = cp.tile([H, H], f32)
            nc.sync.dma_start(W_g1[:], W_g1_d[:])
            W_g2 = cp.tile([H, H], f32)
            nc.sync.dma_start(W_g2[:], W_g2_d[:])
            W_pool = cp.tile([2 * H, H], f32)
            nc.sync.dma_start(W_pool[:], W_pool_d[:])
            W_cls = cp.tile([H, C], f32)
            nc.sync.dma_start(W_cls[:], W_cls_d[:])
            b_pool_c = cp.tile([H, 1], f32)
            nc.sync.dma_start(b_pool_c[:], b_pool_d[:])
            b_cls_c = cp.tile([C, 1], f32)
            nc.sync.dma_start(b_cls_c[:], b_cls_d[:])

            b_bcast = {}
            for nm, bd in [("emb", b_emb_d), ("g1", b_g1_d), ("g2", b_g2_d)]:
                br = cp.tile([1, H], f32, tag=f"brow_{nm}")
                nc.sync.dma_start(br[:], bd[:])
                ps_b = pp.tile([P, H], f32, tag="ps_b", space="PSUM")
                nc.tensor.matmul(ps_b[:], lhsT=ones_row[:], rhs=br[:],
                                 start=True, stop=True)
                bb = cp.tile([P, H], f32, tag=f"bb_{nm}")
                nc.vector.tensor_copy(bb[:], ps_b[:])
                b_bcast[nm] = bb

            deg_all_t = cp.tile([P, NCORES * T], f32)
            nc.sync.dma_start(deg_all_t[:], deg_all_d[:])
            dis_all = cp.tile([P, NCORES * T], f32)
            nc.vector.reciprocal(dis_all[:], deg_all_t[:])
            nc.scalar.activation(dis_all[:], dis_all[:], AF.Sqrt)
            deg_own_t = cp.tile([P, T], f32)
            nc.sync.dma_start(deg_own_t[:], deg_own_d[:])
            dis_own = cp.tile([P, T], f32)
            nc.vector.reciprocal(dis_own[:], deg_own_t[:])
            nc.scalar.activation(dis_own[:], dis_own[:], AF.Sqrt)
            batchl_t = cp.tile([P, T], f32)
            nc.sync.dma_start(batchl_t[:], batchl_d[:])
            padmask_t = cp.tile([P, T], f32)
            nc.sync.dma_start(padmask_t[:], padmask_d[:])

            # ---------------- prologue: u tables from x (primed layout)
            # dest_of(tt) -> (per-slab primed view, local tile col)
            def prologue(xT_src, n_tiles, dis_t, dest_of):
                assert T % WB == 0
                for b0 in range(0, n_tiles, WB):
                    bn = min(WB, n_tiles - b0)
                    ps_slab = pp.tile([P, WB * H], f32, tag="ps_a", space="PSUM")
                    for i in range(bn):
                        tt = b0 + i
                        if tt % SLAB == 0 or i == 0:
                            st0 = tt - tt % SLAB
                            sn = min(SLAB, n_tiles - st0)
                            xsl_cur = sp.tile([D_IN, SLAB * P], f32, tag="xsl")
                            nc.sync.dma_start(
                                xsl_cur[:, :sn * P],
                                xT_src[:, st0 * P:(st0 + sn) * P])
                        nc.tensor.matmul(
                            ps_slab[:, i * H:(i + 1) * H],
                            lhsT=xsl_cur[:, (tt - st0) * P:(tt - st0 + 1) * P],
                            rhs=W_emb[:],
                            start=True, stop=True)
                    s_sl = sp.tile([P, WB * H], f32, tag="s_pro")
                    nc.vector.tensor_tensor(
                        out=s_sl[:, :bn * H].rearrange("p (t f) -> p t f", f=H),
                        in0=ps_slab[:, :bn * H].rearrange("p (t f) -> p t f", f=H),
                        in1=b_bcast["emb"][:].unsqueeze(1).to_broadcast([P, bn, H]),
                        op=ALU.add)
                    r_sl = sp.tile([P, WB * H], f32, tag="r_pro")
                    nc.scalar.activation(r_sl[:, :bn * H], s_sl[:, :bn * H], AF.Relu)
                    u_sl = sp.tile([P, WB * H], f32, tag="u_pro")
                    nc.vector.tensor_tensor(
                        out=u_sl[:, :bn * H].rearrange("p (t f) -> p t f", f=H),
                        in0=r_sl[:, :bn * H].rearrange("p (t f) -> p t f", f=H),
                        in1=dis_t[:, b0:b0 + bn].unsqueeze(2).to_broadcast([P, bn, H]),
                        op=ALU.mult)
                    dview, lcol = dest_of(b0)
                    nc.sync.dma_start(
                        dview[:, lcol * H:(lcol + bn) * H], u_sl[:, :bn * H])

            prologue(xT_all, NCORES * T, dis_all,
                     lambda tt: (u0_tab_slabs[tt // T], tt % T))
            prologue(xT_own, T, dis_own, lambda tt: (u0_own_p, tt))

            def early_out(src_dram):
                tmp = sp.tile([GPC, C], f32, tag="eo")
                nc.sync.dma_start(tmp[:], src_dram[0:GPC, 0:C])
                nc.sync.dma_start(out_d[:], tmp[:])

            # ---------------- conv layers
            ps_sumT = pp1.tile([H, GPC], f32, tag="ps_sumT", space="PSUM")
            h2T_slab = bp.tile([H, S_pad], f32)

            def conv(table, u_own_p, W_L, bb_L, last):
                CP = conv_parts
                for r in runs:
                    rc0, rnc = r["col0"], r["ncols"]
                    if rnc > 0:
                        idx_sl = sp.tile([P, MAXRNC * 8], i16, tag="idx_sl")
                        nc.sync.dma_start(
                            idx_sl[:, :rnc * 8],
                            idx_d[:, rc0 * 8:(rc0 + rnc) * 8])
                        msg = sp.tile([P, MAXRNC * H], f32, tag="msg")
                        if "g" not in CP:
                            nc.gpsimd.memset(msg[:], 0.0)
                        for call in (r["calls"] if "g" in CP else []):
                            q, c0, ncq, NI = (call[kk] for kk in
                                              ["q", "col0", "ncols", "NI"])
                            nrows = min(QROWS, TBL - q * QROWS)
                            nc.gpsimd.dma_gather(
                                out_ap=msg[:, (c0 - rc0) * H:(c0 - rc0 + ncq) * H]
                                    .rearrange("p (g f) -> p g f", f=H),
                                in_ap=table[q * QROWS: q * QROWS + nrows, :],
                                idxs_ap=idx_sl[:, (c0 - rc0) * 8:(c0 - rc0 + ncq) * 8],
                                num_idxs=NI, num_idxs_reg=NI, elem_size=H,
                                single_packet=False)
                        dsl = sp.tile([P, MAXRNC], f32, tag="dsl")
                        nc.sync.dma_start(dsl[:, :rnc], dstl_d[:, rc0:rc0 + rnc])
                    nt = len(r["tiles"])
                    t0 = r["tiles"][0]
                    uo = sp.tile([P, RUN * H], f32, tag="uo")
                    nc.sync.dma_start(uo[:, :nt * H],
                                      u_own_p[:, t0 * H:(t0 + nt) * H])
                    if not last:
                        ubw = sp.tile([P, RUN * H], f32, tag="ubw")
                    for ti, t in enumerate(r["tiles"]):
                        st = S[t]
                        ps_agg = pp.tile([P, H], f32, tag="ps_a", space="PSUM")
                        nc.tensor.matmul(ps_agg[:], lhsT=ident[:],
                                         rhs=uo[:, ti * H:(ti + 1) * H],
                                         start=True,
                                         stop=(st == 0 or "a" not in CP))
                        if st > 0 and "m" in CP:
                            sc0 = sched_of_tile[t] - rc0
                            M_t = sp.tile([P, MAXS * P], f32, tag="M_t")
                            nc.vector.tensor_tensor(
                                out=M_t[:, :st * P].rearrange(
                                    "p (s q) -> p s q", q=P),
                                in0=dsl[:, sc0:sc0 + st].unsqueeze(2)
                                    .to_broadcast([P, st, P]),
                                in1=iota_f[:].unsqueeze(1)
                                    .to_broadcast([P, st, P]),
                                op=ALU.is_equal)
                            if "a" in CP:
                                for j, c in enumerate(gcols[t]):
                                    nc.tensor.matmul(
                                        ps_agg[:],
                                        lhsT=M_t[:, j * P:(j + 1) * P],
                                        rhs=msg[:, (c - rc0) * H:(c - rc0 + 1) * H],
                                        start=False, stop=(j == st - 1))
                        v_t = sp.tile([P, H], f32, tag="v_t")
                        nc.scalar.activation(v_t[:], ps_agg[:], AF.Copy,
                                             scale=dis_own[:, t:t + 1])
                        ps_vt = pp.tile([H, P], f32, tag="ps_b", space="PSUM")
                        nc.tensor.transpose(ps_vt[:], v_t[:], ident[:])
                        vt_s = sp.tile([H, P], f32, tag="vt_s")
                        nc.vector.tensor_copy(vt_s[:], ps_vt[:])
                        ps_o = pp.tile([P, H], f32, tag="ps_o", space="PSUM")
                        nc.tensor.matmul(ps_o[:], lhsT=vt_s[:], rhs=W_L[:],
                                         start=True, stop=True)
                        s2 = sp.tile([P, H], f32, tag="s2")
                        nc.vector.tensor_tensor(out=s2[:], in0=ps_o[:],
                                                in1=bb_L[:], op=ALU.add)
                        if not last:
                            nc.scalar.activation(ubw[:, ti * H:(ti + 1) * H],
                                                 s2[:], AF.Relu,
                                                 scale=dis_own[:, t:t + 1])
                        else:
                            h2 = sp.tile([P, H], f32, tag="h2")
                            nc.scalar.activation(h2[:], s2[:], AF.Relu,
                                                 scale=padmask_t[:, t:t + 1])
                            B_t = sp.tile([P, GPC], f32, tag="B_t")
                            nc.vector.tensor_tensor(
                                out=B_t[:],
                                in0=batchl_t[:, t:t + 1].to_broadcast([P, GPC]),
                                in1=iota16_f[:], op=ALU.is_equal)
                            nc.tensor.matmul(ps_sumT[:], lhsT=h2[:], rhs=B_t[:],
                                             start=(t == 0), stop=(t == T - 1))
                            ps_h2t = pp.tile([H, P], f32, tag="ps_b", space="PSUM")
                            nc.tensor.transpose(ps_h2t[:], h2[:], ident[:])
                            nc.vector.tensor_copy(
                                h2T_slab[:, t * P:(t + 1) * P], ps_h2t[:])
                    if not last:
                        nc.sync.dma_start(
                            u1_shard_p[:, t0 * H:(t0 + nt) * H],
                            ubw[:, :nt * H])

            if stage == 1:
                early_out(u0_tab)
            if stage >= 2:
                conv(u0_tab, u0_own_p, W_g1, b_bcast["g1"], last=False)
                if stage == 2:
                    early_out(u1_shard)
            if stage >= 3:
                nc.gpsimd.collective_compute(
                    "AllGather", ALU.bypass,
                    replica_groups=[list(range(NCORES))],
                    ins=[u1_shard[:]], outs=[u1_tab[:]])
                if stage == 3:
                    early_out(u1_tab)
            if stage >= 4:
                conv(u1_tab, u1_shard_p, W_g2, b_bcast["g2"], last=True)
                if stage == 4:
                    early_out(u1_tab)

            if stage >= 5:
                    # ---------------- head
                cnt_t = cp.tile([1, GPC], f32)
                nc.sync.dma_start(cnt_t[:], cnt_d[:])
                invc = cp.tile([1, GPC], f32)
                nc.vector.reciprocal(invc[:], cnt_t[:])
                ps_ic = pp.tile([H, GPC], f32, tag="ps_b", space="PSUM")
                nc.tensor.matmul(ps_ic[:], lhsT=ones_row[:, :H], rhs=invc[:],
                                 start=True, stop=True)
                ic_s = sp.tile([H, GPC], f32, tag="ic_s")
                nc.vector.tensor_copy(ic_s[:], ps_ic[:])
                meanT = sp.tile([H, GPC], f32, tag="meanT")
                nc.vector.tensor_tensor(out=meanT[:], in0=ps_sumT[:], in1=ic_s[:],
                                        op=ALU.mult)
                maxT = sp.tile([H, GPC], f32, tag="maxT")
                for g in range(GPC):
                    nc.vector.reduce_max(
                        maxT[:, g:g + 1],
                        h2T_slab[:, g * TG * P:(g + 1) * TG * P],
                        axis=AX.X)
                cat_s = sp.tile([P, GPC], f32, tag="cat_s")
                nc.sync.dma_start(cat_s[0:H, :], meanT[:])
                nc.sync.dma_start(cat_s[H:2 * H, :], maxT[:])
                ps_hg = pp.tile([H, GPC], f32, tag="ps_b", space="PSUM")
                nc.tensor.matmul(ps_hg[:], lhsT=W_pool[:], rhs=cat_s[:],
                                 start=True, stop=True)
                hg_s = sp.tile([H, GPC], f32, tag="hg_s")
                nc.vector.tensor_tensor(out=hg_s[:], in0=ps_hg[:],
                                        in1=b_pool_c[:].to_broadcast([H, GPC]),
                                        op=ALU.add)
                ps_lg = pp.tile([C, GPC], f32, tag="ps_b", space="PSUM")
                nc.tensor.matmul(ps_lg[:], lhsT=W_cls[:], rhs=hg_s[:],
                                 start=True, stop=True)
                lg_s = sp.tile([C, GPC], f32, tag="lg_s")
                nc.vector.tensor_tensor(out=lg_s[:], in0=ps_lg[:],
                                        in1=b_cls_c[:].to_broadcast([C, GPC]),
                                        op=ALU.add)
                ps_z = pp.tile([GPC, C], f32, tag="ps_b", space="PSUM")
                nc.tensor.transpose(ps_z[:], lg_s[:], ident[0:C, 0:C])
                z = sp.tile([GPC, C], f32, tag="z")
                nc.vector.tensor_copy(z[:], ps_z[:])
                zm = sp.tile([GPC, 1], f32, tag="zm")
                nc.vector.reduce_max(zm[:], z[:], axis=AX.X)
                zs = sp.tile([GPC, C], f32, tag="zs")
                nc.vector.tensor_tensor(out=zs[:], in0=z[:],
                                        in1=zm[:].to_broadcast([GPC, C]),
                                        op=ALU.subtract)
                ez = sp.tile([GPC, C], f32, tag="ez")
                nc.scalar.activation(ez[:], zs[:], AF.Exp)
                es = sp.tile([GPC, 1], f32, tag="es")
                nc.vector.reduce_sum(es[:], ez[:], axis=AX.X)
                les = sp.tile([GPC, 1], f32, tag="les")
                nc.scalar.activation(les[:], es[:], AF.Ln)
                res = sp.tile([GPC, C], f32, tag="res")
                nc.vector.tensor_tensor(out=res[:], in0=zs[:],
                                        in1=les[:].to_broadcast([GPC, C]),
                                        op=ALU.subtract)
                nc.sync.dma_start(out_d[:], res[:])

    nc.finalize()
    return nc


# ----------------------------------------------------------------------------
# entry point
# ----------------------------------------------------------------------------

_trace = {"on": False, "res": None}


def kernel(**inputs):
    from concourse.bass_utils import run_bass_kernel_spmd

    x = np.asarray(inputs["x"], np.float32)
    src = np.asarray(inputs["src"])
    dst = np.asarray(inputs["dst"])
    batch = np.asarray(inputs["batch"])

    meta, per_core = build_meta(src, dst, batch)

    xT_all, _ = pack_xT4(x, meta)
    common = dict(
        xT_all=xT_all,
        deg_all=meta["deg_all"],
        W_emb=np.asarray(inputs["W_emb"], np.float32),
        W_g1=np.asarray(inputs["W_g1"], np.float32),
        W_g2=np.asarray(inputs["W_g2"], np.float32),
        W_pool=np.asarray(inputs["W_pool"], np.float32),
        W_cls=np.asarray(inputs["W_cls"], np.float32),
        b_emb_r=np.asarray(inputs["b_emb"], np.float32).reshape(1, H),
        b_g1_r=np.asarray(inputs["b_g1"], np.float32).reshape(1, H),
        b_g2_r=np.asarray(inputs["b_g2"], np.float32).reshape(1, H),
        b_pool_c=np.asarray(inputs["b_pool"], np.float32).reshape(H, 1),
        b_cls_c=np.asarray(inputs["b_cls"], np.float32).reshape(C, 1),
    )
    in_maps = []
    for k in range(NCORES):
        pc = per_core[k]
        xT_own, _ = pack_xT4(x, meta, core=k)
        in_maps.append(dict(
            common,
            xT_own=xT_own,
            deg_own=pc["deg_own"],
            idx16=pc["idx16"],
            dstl=pc["dstl"],
            batchl=pc["batch_local"],
            padmask=pc["padmask"],
            cntg=pc["cnt"],
        ))

    nc = build_program(meta, stage=_trace.get("stage", 5),
                       conv_parts=_trace.get("conv_parts", "gmap"))
    _trace["nc"] = nc
    _trace["in_maps"] = in_maps
    res = run_bass_kernel_spmd(
        nc, in_maps, core_ids=list(range(NCORES)),
        trace=_trace["on"])
    _trace["res"] = res
    out = np.concatenate([res.results[k]["out"] for k in range(NCORES)], axis=0)
    return out.astype(np.float32)



# revision 4
# speedup vs baseline: 77.3334x; 77.3334x over previous
"""Trainium2 Bass kernel for nn_CascadeGNN (2-layer GCN + mean/max pool + cls).

Strategy (8 NeuronCores, data-parallel over graphs):
  - Nodes/edges sharded by graph id (batch is sorted -> contiguous shards,
    16 graphs per core). Each graph gets a fixed slot of TG node tiles so the
    SPMD program is uniform across cores. Edges live on the core owning dst.
  - Key identity: with u = dis * h, a GCN layer is
        h' = relu(dis * (sum_{e: src->n} u[src] + u[n]) @ W + b)
    so cores exchange only the small u tables (AllGather) and apply W
    post-aggregation. Layer-0 u is likewise computed per-shard and gathered.
  - Per 128-node tile, edge messages are gathered with dma_gather (bulk SWDGE
    gather, int16 indices -> the padded table is split in <=32767-row
    quarters) and reduced on the TensorEngine via one-hot matrices
    M[e, n] = (dst_local[e] == n) built on the VectorEngine (iota+is_equal).
  - All graph-structure data (gather indices, dst labels, degree/pool masks)
    is baked into the NEFF as Const tensors holding all 8 cores' shards;
    at run start each core extracts its own shard with a partition-id-
    dependent dma_gather.  Per-call inputs are only the bf16-packed node
    features (+ W_emb) and a packed f32 weight vector -> ~0.26 MB/core.
  - Pooling: mean via per-tile matmul against a premultiplied pad/count
    column; max via per-tile transpose + running reduce_max.

The Bass program is compiled per graph structure (edge schedule baked in)
and cached, along with a jitted PJRT runner, across kernel() calls.
"""
import hashlib
import numpy as np
import ml_dtypes

P = 128
NCORES = 8
H = 64
D_IN = 8
RUN = 4
GPC = 16

N = 100000
E = 1600000
G = 128
C = 2

BF16 = ml_dtypes.bfloat16

MISC_W = 512          # misc blob row width (f32): dis | padmask | poolw | spare
WPACK = 4096 + 64 + 4096 + 64 + 8192 + 64 + 128 + 2 + 64  # packed f32 weights


# ----------------------------------------------------------------------------
# host-side metadata (sharding / index prep)
# ----------------------------------------------------------------------------

def build_meta(src, dst, batch):
    graph_start = np.searchsorted(batch, np.arange(G + 1))
    gsizes = (graph_start[1:] - graph_start[:-1]).astype(np.int64)
    TG = int(np.ceil(max(int(gsizes.max()), 1) / P))
    T = GPC * TG
    S_pad = T * P
    TBL = NCORES * S_pad
    NQ = int(np.ceil(TBL / 32767.0))
    QROWS = int(np.ceil(TBL / NQ / P)) * P

    # node -> padded table row (logical: local = tile*128 + partition)
    map_row = np.empty(N, np.int64)
    for g in range(G):
        k, slot = g // GPC, g % GPC
        a, b = graph_start[g], graph_start[g + 1]
        map_row[a:b] = k * S_pad + slot * TG * P + np.arange(b - a)

    deg = np.bincount(dst, minlength=N).astype(np.float64) + 1.0
    dis = (1.0 / np.sqrt(deg)).astype(np.float32)

    order = np.argsort(dst, kind="stable")
    src_s = src[order].astype(np.int64)
    dst_s = dst[order].astype(np.int64)
    # primed (partition-major) table row of the source
    sr = map_row[src_s]
    sk, sloc = sr // S_pad, sr % S_pad
    src_rowp = sk * S_pad + (sloc % P) * T + (sloc // P)
    src_q = src_rowp // QROWS
    src_rel = (src_rowp - src_q * QROWS).astype(np.int64)
    dst_row = map_row[dst_s]

    buckets = {}
    cnt = np.zeros((NCORES, T, NQ), np.int64)
    for k in range(NCORES):
        e0 = np.searchsorted(dst_row, k * S_pad)
        e1 = np.searchsorted(dst_row, (k + 1) * S_pad)
        loc = dst_row[e0:e1] - k * S_pad
        tq = loc // P
        t_start = e0 + np.searchsorted(tq, np.arange(T + 1))
        for t in range(T):
            a, b = t_start[t], t_start[t + 1]
            q_e = src_q[a:b]
            loc_t = loc[a - e0:b - e0] - t * P
            for q in range(NQ):
                m = q_e == q
                buckets[(k, t, q)] = (src_rel[a:b][m], loc_t[m])
                cnt[k, t, q] = int(m.sum())

    Gtq = (-(-cnt // P)).max(axis=0)

    n_runs = int(np.ceil(T / RUN))
    run_tiles = [list(range(r * RUN, min((r + 1) * RUN, T))) for r in range(n_runs)]
    runs = []
    col = 0
    sec_col = {}
    gcols = [[] for _ in range(T)]
    for tiles in run_tiles:
        run_col0 = col
        calls = []
        for q in range(NQ):
            ncols_q = int(sum(Gtq[t, q] for t in tiles))
            if ncols_q == 0:
                continue
            q_col0 = col
            for t in tiles:
                sec_col[(t, q)] = (col, int(Gtq[t, q]))
                gcols[t].extend(range(col, col + int(Gtq[t, q])))
                col += int(Gtq[t, q])
            calls.append(dict(q=q, col0=q_col0, ncols=ncols_q, NI=ncols_q * P))
        runs.append(dict(tiles=tiles, col0=run_col0, ncols=col - run_col0,
                         calls=calls))
    NCOL = col
    NSLOT = NCOL * P
    NSLOT16P = -(-(NCOL * 8) // P) * P      # idx blob row width (i16 elems)
    DSTL_ROW = -(-NCOL // 256) * 256        # dstl blob row width (i8)
    S = [len(g) for g in gcols]
    sched_of_tile = {}
    sc = 0
    for r in runs:
        for t in r["tiles"]:
            sched_of_tile[t] = sc
            sc += S[t]
    assert sc == NCOL

    def to_slot_layout(vals_per_node, pad_value, k):
        out = np.full(S_pad, pad_value, np.float32)
        for g in range(k * GPC, (k + 1) * GPC):
            a, b = graph_start[g], graph_start[g + 1]
            slot = g % GPC
            out[slot * TG * P: slot * TG * P + (b - a)] = vals_per_node[a:b]
        return out.reshape(T, P).T.copy()

    inv_cnt_node = (1.0 / np.maximum(gsizes, 1)[batch]).astype(np.float32)

    idx_all = np.zeros((NCORES * 16, NSLOT16P), np.int16)
    dstl_all = np.full((NCORES * P, DSTL_ROW), -1, np.int8)
    misc_all = np.zeros((NCORES * P, MISC_W), np.float32)
    for k in range(NCORES):
        idx_lin = np.zeros(NSLOT, np.int16)
        slot_dl = np.full(NSLOT, -1, np.int64)
        for t in range(T):
            for q in range(NQ):
                if (t, q) not in sec_col:
                    continue
                c0, nc_ = sec_col[(t, q)]
                if nc_ == 0:
                    continue
                rel, dl = buckets[(k, t, q)]
                n = len(rel)
                off = c0 * P
                idx_lin[off:off + n] = rel.astype(np.int16)
                slot_dl[off:off + n] = dl
        idx_all[k * 16:(k + 1) * 16, :NSLOT // 16] = \
            idx_lin.reshape(NSLOT // 16, 16).T
        for t in range(T):
            sc0 = sched_of_tile[t]
            for j, c in enumerate(gcols[t]):
                sd = slot_dl[c * P:(c + 1) * P]
                dstl_all[k * P:(k + 1) * P, sc0 + j] = \
                    np.where(sd >= 0, sd, -1).astype(np.int8)
        misc_all[k * P:(k + 1) * P, 0:T] = to_slot_layout(dis, 0.0, k)
        misc_all[k * P:(k + 1) * P, 128:128 + T] = \
            to_slot_layout(np.ones(N, np.float32), 0.0, k)
        misc_all[k * P:(k + 1) * P, 256:256 + T] = \
            to_slot_layout(inv_cnt_node, 0.0, k)

    MAXS = max(max(S), 1)
    MAXRNC = max((r["ncols"] for r in runs), default=1)

    return dict(
        T=T, TG=TG, S_pad=S_pad, TBL=TBL, NQ=NQ, QROWS=QROWS,
        NCOL=NCOL, NSLOT=NSLOT, NSLOT16P=NSLOT16P, DSTL_ROW=DSTL_ROW,
        runs=runs, gcols=gcols, S=S, sched_of_tile=sched_of_tile,
        MAXS=MAXS, MAXRNC=MAXRNC,
        graph_start=graph_start, map_row=map_row, gsizes=gsizes,
        idx_all=idx_all, dstl_all=dstl_all, misc_all=misc_all,
    )


def pack_weights(inputs):
    parts = [
        np.asarray(inputs["W_g1"], np.float32).reshape(-1),
        np.asarray(inputs["b_g1"], np.float32).reshape(-1),
        np.asarray(inputs["W_g2"], np.float32).reshape(-1),
        np.asarray(inputs["b_g2"], np.float32).reshape(-1),
        np.asarray(inputs["W_pool"], np.float32).reshape(-1),
        np.asarray(inputs["b_pool"], np.float32).reshape(-1),
        np.asarray(inputs["W_cls"], np.float32).reshape(-1),
        np.asarray(inputs["b_cls"], np.float32).reshape(-1),
        np.asarray(inputs["b_emb"], np.float32).reshape(-1),
    ]
    w = np.concatenate(parts)
    assert w.size == WPACK, w.size
    return w.reshape(1, WPACK)


# ----------------------------------------------------------------------------
# device program
# ----------------------------------------------------------------------------

def build_program(meta, stage=5):
    import concourse.mybir as mybir
    import concourse.tile as tile
    from concourse import bacc
    from concourse.masks import make_identity

    f32 = mybir.dt.float32
    bf16 = mybir.dt.bfloat16
    i16 = mybir.dt.int16
    i32 = mybir.dt.int32
    i8 = mybir.dt.int8
    u32 = mybir.dt.uint32
    AF = mybir.ActivationFunctionType
    ALU = mybir.AluOpType
    AX = mybir.AxisListType

    T, TG, S_pad, TBL, NQ, QROWS, NCOL, NSLOT = (meta[k] for k in
        ["T", "TG", "S_pad", "TBL", "NQ", "QROWS", "NCOL", "NSLOT"])
    NSLOT16P, DSTL_ROW = meta["NSLOT16P"], meta["DSTL_ROW"]
    runs, gcols, S, sched_of_tile = (meta[k] for k in
        ["runs", "gcols", "S", "sched_of_tile"])
    MAXS, MAXRNC = meta["MAXS"], meta["MAXRNC"]
    SLAB = 16  # tiles per xT slab
    WB = 8     # tiles per prologue write batch (one PSUM bank: 8*64=512 f32)

    nc = bacc.Bacc("TRN2", target_bir_lowering=False)

    xT_d = nc.dram_tensor("xT_in", [D_IN, S_pad + H], bf16, kind="ExternalInput")
    wpack_d = nc.dram_tensor("wpack", [1, WPACK], f32, kind="ExternalInput")
    out_d = nc.dram_tensor("out", [GPC, C], f32, kind="ExternalOutput")

    idx_all_d = nc.inline_tensor(meta["idx_all"], "idx_all")
    dstl_all_d = nc.inline_tensor(meta["dstl_all"], "dstl_all")
    misc_all_d = nc.inline_tensor(meta["misc_all"], "misc_all")

    u0_shard = nc.dram_tensor("u0_shard", [S_pad, H], f32)
    u0_tab = nc.dram_tensor("u0_tab", [TBL, H], f32)
    u1_shard = nc.dram_tensor("u1_shard", [S_pad, H], f32)
    u1_tab = nc.dram_tensor("u1_tab", [TBL, H], f32)

    # primed views: [P, T*H] (partition p, tile-major contiguous)
    def primed(tensor):
        return tensor[:, :].rearrange("(p c) f -> p (c f)", p=P)

    u0_shard_p = primed(u0_shard)
    u1_shard_p = primed(u1_shard)

    # packed-weight offsets
    WOFF = {}
    off = 0
    for nm, sz in [("W_g1", H * H), ("b_g1", H), ("W_g2", H * H), ("b_g2", H),
                   ("W_pool", 2 * H * H), ("b_pool", H), ("W_cls", H * C),
                   ("b_cls", C), ("b_emb", H)]:
        WOFF[nm] = (off, sz)
        off += sz

    def wview(nm, r, c):
        a, sz = WOFF[nm]
        assert sz == r * c
        return wpack_d[0:1, a:a + sz].rearrange("o (r c) -> (o r) c", c=c)

    with tile.TileContext(nc) as tc:
        with (
            tc.tile_pool(name="psum", bufs=2, space="PSUM") as pp,
            tc.tile_pool(name="psum1", bufs=1, space="PSUM") as pp1,
            tc.tile_pool(name="const", bufs=1) as cp,
        ):
            # ---------------- constants
            ident = cp.tile([P, P], f32)
            make_identity(nc, ident[:])
            iota_i = cp.tile([P, P], i32)
            nc.gpsimd.iota(iota_i[:], pattern=[[1, P]], base=0, channel_multiplier=0)
            iota_f = cp.tile([P, P], f32)
            nc.vector.tensor_copy(iota_f[:], iota_i[:])
            ones_row = cp.tile([1, P], f32)
            nc.gpsimd.memset(ones_row[:], 1.0)

            W_emb = cp.tile([D_IN, H], bf16)
            nc.sync.dma_start(W_emb[:], xT_d[:, S_pad:S_pad + H])
            W_g1 = cp.tile([H, H], f32)
            nc.sync.dma_start(W_g1[:], wview("W_g1", H, H))
            W_g2 = cp.tile([H, H], f32)
            nc.sync.dma_start(W_g2[:], wview("W_g2", H, H))
            W_pool = cp.tile([2 * H, H], f32)
            nc.sync.dma_start(W_pool[:], wview("W_pool", 2 * H, H))
            W_cls = cp.tile([H, C], f32)
            nc.sync.dma_start(W_cls[:], wview("W_cls", H, C))
            b_pool_c = cp.tile([H, 1], f32)
            nc.sync.dma_start(b_pool_c[:], wview("b_pool", H, 1))
            b_cls_c = cp.tile([C, 1], f32)
            nc.sync.dma_start(b_cls_c[:], wview("b_cls", C, 1))

            b_bcast = {}
            for nm in ["b_emb", "b_g1", "b_g2"]:
                br = cp.tile([1, H], f32, tag=f"brow_{nm}")
                nc.sync.dma_start(br[:], wview(nm, 1, H))
                ps_b = pp.tile([P, H], f32, tag="ps_b", space="PSUM")
                nc.tensor.matmul(ps_b[:], lhsT=ones_row[:], rhs=br[:],
                                 start=True, stop=True)
                bb = cp.tile([P, H], f32, tag=f"bb_{nm}")
                nc.vector.tensor_copy(bb[:], ps_b[:])
                b_bcast[nm] = bb

            # ---------------- partition-id machinery + per-core const fetch
            pid_u = cp.tile([1, 1], u32, tag="pid_u")
            nc.sync.dma_start(pid_u[:], nc.partition_id_tensor[0:1, 0:1])
            pid_f = cp.tile([1, 1], f32, tag="pid_f")
            nc.vector.tensor_copy(pid_f[:], pid_u[:])
            ps_pid = pp.tile([P, 1], f32, tag="ps_b", space="PSUM")
            nc.tensor.matmul(ps_pid[:], lhsT=ones_row[:], rhs=pid_f[:],
                             start=True, stop=True)
            pid_col = cp.tile([P, 1], f32, tag="pid_col")
            nc.vector.tensor_copy(pid_col[:], ps_pid[:])

            # p%16 column and 16*c row iotas as f32
            pm_i = cp.tile([P, 1], i32, tag="pm_i")
            nc.gpsimd.iota(pm_i[:], pattern=[[1, 1]], base=0, channel_multiplier=1)
            nc.vector.tensor_scalar(out=pm_i[:], in0=pm_i[:], scalar1=15,
                                    scalar2=None, op0=ALU.bitwise_and)
            pm_f = cp.tile([P, 1], f32, tag="pm_f")
            nc.vector.tensor_copy(pm_f[:], pm_i[:])
            c16_i = cp.tile([P, 8], i32, tag="c16_i")
            nc.gpsimd.iota(c16_i[:], pattern=[[16, 8]], base=0, channel_multiplier=0)
            c16_f = cp.tile([P, 8], f32, tag="c16_f")
            nc.vector.tensor_copy(c16_f[:], c16_i[:])

            def pid_idx(tag, scale, with_c16):
                # int16 [P, 8] gather indices: scale*pid + p%16 (+ 16c)
                sc = cp.tile([P, 1], f32, tag=f"{tag}_sc")
                nc.vector.tensor_scalar(out=sc[:], in0=pid_col[:], scalar1=float(scale),
                                        scalar2=None, op0=ALU.mult)
                f = cp.tile([P, 8], f32, tag=f"{tag}_f")
                nc.vector.tensor_scalar(out=f[:], in0=pm_f[:].to_broadcast([P, 8]),
                                        scalar1=sc[:], scalar2=None, op0=ALU.add)
                if with_c16:
                    nc.vector.tensor_tensor(out=f[:], in0=f[:], in1=c16_f[:],
                                            op=ALU.add)
                ix = cp.tile([P, 8], i16, tag=f"{tag}_i")
                nc.vector.tensor_copy(ix[:], f[:])
                return ix

            idxA = pid_idx("idxA", 16, with_c16=False)   # idx blob: 16*pid + p%16
            idxB = pid_idx("idxB", 128, with_c16=True)   # row blobs: 128*pid + i

            idx_res = cp.tile([P, NSLOT16P], i16, tag="idx_res")
            nc.gpsimd.dma_gather(
                out_ap=idx_res[:].rearrange("p (g f) -> p g f", f=NSLOT16P),
                in_ap=idx_all_d[:, :],
                idxs_ap=idxA[:],
                num_idxs=P, num_idxs_reg=P, elem_size=NSLOT16P,
                single_packet=False)
            misc_t = cp.tile([P, MISC_W], f32, tag="misc_t")
            nc.gpsimd.dma_gather(
                out_ap=misc_t[:].rearrange("p (g f) -> p g f", f=MISC_W),
                in_ap=misc_all_d[:, :],
                idxs_ap=idxB[:],
                num_idxs=P, num_idxs_reg=P, elem_size=MISC_W,
                single_packet=False)
            dstl_raw = cp.tile([P, DSTL_ROW], i8, tag="dstl_raw")
            nc.gpsimd.dma_gather(
                out_ap=dstl_raw[:].rearrange("p (g f) -> p g f", f=DSTL_ROW),
                in_ap=dstl_all_d[:, :],
                idxs_ap=idxB[:],
                num_idxs=P, num_idxs_reg=P, elem_size=DSTL_ROW,
                single_packet=False)
            dstl_f = cp.tile([P, NCOL], f32, tag="dstl_f")
            nc.vector.tensor_copy(dstl_f[:], dstl_raw[:, :NCOL])

            dis_own = misc_t[:, 0:T]
            padmask = misc_t[:, 128:128 + T]
            poolw = misc_t[:, 256:256 + T]

            with (
                tc.tile_pool(name="sbuf", bufs=2) as sp,
            ):
                # ---------------- prologue: u0 for own shard (primed layout)
                assert T % WB == 0
                for b0 in range(0, T, WB):
                    bn = min(WB, T - b0)
                    ps_slab = pp.tile([P, WB * H], f32, tag="ps_a", space="PSUM")
                    for i in range(bn):
                        tt = b0 + i
                        if tt % SLAB == 0 or i == 0:
                            st0 = tt - tt % SLAB
                            sn = min(SLAB, T - st0)
                            xsl_cur = sp.tile([D_IN, SLAB * P], bf16, tag="xsl")
                            nc.sync.dma_start(
                                xsl_cur[:, :sn * P],
                                xT_d[:, st0 * P:(st0 + sn) * P])
                        nc.tensor.matmul(
                            ps_slab[:, i * H:(i + 1) * H],
                            lhsT=xsl_cur[:, (tt - st0) * P:(tt - st0 + 1) * P],
                            rhs=W_emb[:],
                            start=True, stop=True)
                    s_sl = sp.tile([P, WB * H], f32, tag="s_pro")
                    nc.vector.tensor_tensor(
                        out=s_sl[:, :bn * H].rearrange("p (t f) -> p t f", f=H),
                        in0=ps_slab[:, :bn * H].rearrange("p (t f) -> p t f", f=H),
                        in1=b_bcast["b_emb"][:].unsqueeze(1).to_broadcast([P, bn, H]),
                        op=ALU.add)
                    r_sl = sp.tile([P, WB * H], f32, tag="r_pro")
                    nc.scalar.activation(r_sl[:, :bn * H], s_sl[:, :bn * H], AF.Relu)
                    u_sl = sp.tile([P, WB * H], f32, tag="u_pro")
                    nc.vector.tensor_tensor(
                        out=u_sl[:, :bn * H].rearrange("p (t f) -> p t f", f=H),
                        in0=r_sl[:, :bn * H].rearrange("p (t f) -> p t f", f=H),
                        in1=dis_own[:, b0:b0 + bn].unsqueeze(2).to_broadcast([P, bn, H]),
                        op=ALU.mult)
                    nc.sync.dma_start(
                        u0_shard_p[:, b0 * H:(b0 + bn) * H], u_sl[:, :bn * H])

                def early_out(src_dram):
                    tmp = sp.tile([GPC, C], f32, tag="eo")
                    nc.sync.dma_start(tmp[:], src_dram[0:GPC, 0:C])
                    nc.sync.dma_start(out_d[:], tmp[:])

                def allgather(src, dst):
                    nc.gpsimd.collective_compute(
                        "AllGather", ALU.bypass,
                        replica_groups=[list(range(NCORES))],
                        ins=[src[:]], outs=[dst[:]])

                # ---------------- conv layers
                ps_sumT = pp1.tile([H, GPC], f32, tag="ps_sumT", space="PSUM")
                maxT = cp.tile([H, GPC], f32, tag="maxT")
                nc.gpsimd.memset(maxT[:], 0.0)
                meanT = cp.tile([H, GPC], f32, tag="meanT")

                def conv(table, u_own_p, W_L, bb_L, last):
                    for r in runs:
                        rc0, rnc = r["col0"], r["ncols"]
                        if rnc > 0:
                            msg = sp.tile([P, MAXRNC * H], f32, tag="msg")
                            for call in r["calls"]:
                                q, c0, ncq, NI = (call[kk] for kk in
                                                  ["q", "col0", "ncols", "NI"])
                                nrows = min(QROWS, TBL - q * QROWS)
                                nc.gpsimd.dma_gather(
                                    out_ap=msg[:, (c0 - rc0) * H:(c0 - rc0 + ncq) * H]
                                        .rearrange("p (g f) -> p g f", f=H),
                                    in_ap=table[q * QROWS: q * QROWS + nrows, :],
                                    idxs_ap=idx_res[:, c0 * 8:(c0 + ncq) * 8],
                                    num_idxs=NI, num_idxs_reg=NI, elem_size=H,
                                    single_packet=False)
                        nt = len(r["tiles"])
                        t0 = r["tiles"][0]
                        uo = sp.tile([P, RUN * H], f32, tag="uo")
                        nc.sync.dma_start(uo[:, :nt * H],
                                          u_own_p[:, t0 * H:(t0 + nt) * H])
                        if not last:
                            ubw = sp.tile([P, RUN * H], f32, tag="ubw")
                        for ti, t in enumerate(r["tiles"]):
                            st = S[t]
                            ps_agg = pp.tile([P, H], f32, tag="ps_a", space="PSUM")
                            nc.tensor.matmul(ps_agg[:], lhsT=ident[:],
                                             rhs=uo[:, ti * H:(ti + 1) * H],
                                             start=True, stop=(st == 0))
                            if st > 0:
                                sc0 = sched_of_tile[t]
                                M_t = sp.tile([P, MAXS * P], f32, tag="M_t")
                                nc.vector.tensor_tensor(
                                    out=M_t[:, :st * P].rearrange(
                                        "p (s q) -> p s q", q=P),
                                    in0=dstl_f[:, sc0:sc0 + st].unsqueeze(2)
                                        .to_broadcast([P, st, P]),
                                    in1=iota_f[:].unsqueeze(1)
                                        .to_broadcast([P, st, P]),
                                    op=ALU.is_equal)
                                for j, c in enumerate(gcols[t]):
                                    nc.tensor.matmul(
                                        ps_agg[:],
                                        lhsT=M_t[:, j * P:(j + 1) * P],
                                        rhs=msg[:, (c - rc0) * H:(c - rc0 + 1) * H],
                                        start=False, stop=(j == st - 1))
                            v_t = sp.tile([P, H], f32, tag="v_t")
                            nc.scalar.activation(v_t[:], ps_agg[:], AF.Copy,
                                                 scale=dis_own[:, t:t + 1])
                            ps_vt = pp.tile([H, P], f32, tag="ps_b", space="PSUM")
                            nc.tensor.transpose(ps_vt[:], v_t[:], ident[:])
                            vt_s = sp.tile([H, P], f32, tag="vt_s")
                            nc.vector.tensor_copy(vt_s[:], ps_vt[:])
                            ps_o = pp.tile([P, H], f32, tag="ps_o", space="PSUM")
                            nc.tensor.matmul(ps_o[:], lhsT=vt_s[:], rhs=W_L[:],
                                             start=True, stop=True)
                            s2 = sp.tile([P, H], f32, tag="s2")
                            nc.vector.tensor_tensor(out=s2[:], in0=ps_o[:],
                                                    in1=bb_L[:], op=ALU.add)
                            if not last:
                                nc.scalar.activation(ubw[:, ti * H:(ti + 1) * H],
                                                     s2[:], AF.Relu,
                                                     scale=dis_own[:, t:t + 1])
                            else:
                                g = t // TG
                                h2 = sp.tile([P, H], f32, tag="h2")
                                nc.scalar.activation(h2[:], s2[:], AF.Relu,
                                                     scale=padmask[:, t:t + 1])
                                nc.tensor.matmul(ps_sumT[:, g:g + 1], lhsT=h2[:],
                                                 rhs=poolw[:, t:t + 1],
                                                 start=(t % TG == 0),
                                                 stop=(t % TG == TG - 1))
                                ps_h2t = pp.tile([H, P], f32, tag="ps_b",
                                                 space="PSUM")
                                nc.tensor.transpose(ps_h2t[:], h2[:], ident[:])
                                tmax = sp.tile([H, 1], f32, tag="tmax")
                                nc.vector.reduce_max(tmax[:], ps_h2t[:], axis=AX.X)
                                nc.vector.tensor_tensor(
                                    out=maxT[:, g:g + 1], in0=maxT[:, g:g + 1],
                                    in1=tmax[:], op=ALU.max)
                        if not last:
                            nc.sync.dma_start(
                                u1_shard_p[:, t0 * H:(t0 + nt) * H],
                                ubw[:, :nt * H])

                allgather(u0_shard, u0_tab)
                if stage == 1:
                    early_out(u0_tab)
                if stage >= 2:
                    conv(u0_tab, u0_shard_p, W_g1, b_bcast["b_g1"], last=False)
                    if stage == 2:
                        early_out(u1_shard)
                if stage >= 3:
                    allgather(u1_shard, u1_tab)
                    if stage == 3:
                        early_out(u1_tab)
                if stage >= 4:
                    conv(u1_tab, u1_shard_p, W_g2, b_bcast["b_g2"], last=True)
                    if stage == 4:
                        early_out(u1_tab)

                if stage >= 5:
                    # ---------------- head
                    nc.vector.tensor_copy(meanT[:], ps_sumT[:])
                    cat_s = sp.tile([P, GPC], f32, tag="cat_s")
                    nc.sync.dma_start(cat_s[0:H, :], meanT[:])
                    nc.sync.dma_start(cat_s[H:2 * H, :], maxT[:])
                    ps_hg = pp.tile([H, GPC], f32, tag="ps_b", space="PSUM")
                    nc.tensor.matmul(ps_hg[:], lhsT=W_pool[:], rhs=cat_s[:],
                                     start=True, stop=True)
                    hg_s = sp.tile([H, GPC], f32, tag="hg_s")
                    nc.vector.tensor_tensor(out=hg_s[:], in0=ps_hg[:],
                                            in1=b_pool_c[:].to_broadcast([H, GPC]),
                                            op=ALU.add)
                    ps_lg = pp.tile([C, GPC], f32, tag="ps_b", space="PSUM")
                    nc.tensor.matmul(ps_lg[:], lhsT=W_cls[:], rhs=hg_s[:],
                                     start=True, stop=True)
                    lg_s = sp.tile([C, GPC], f32, tag="lg_s")
                    nc.vector.tensor_tensor(out=lg_s[:], in0=ps_lg[:],
                                            in1=b_cls_c[:].to_broadcast([C, GPC]),
                                            op=ALU.add)
                    ps_z = pp.tile([GPC, C], f32, tag="ps_b", space="PSUM")
                    nc.tensor.transpose(ps_z[:], lg_s[:], ident[0:C, 0:C])
                    z = sp.tile([GPC, C], f32, tag="z")
                    nc.vector.tensor_copy(z[:], ps_z[:])
                    zm = sp.tile([GPC, 1], f32, tag="zm")
                    nc.vector.reduce_max(zm[:], z[:], axis=AX.X)
                    zs = sp.tile([GPC, C], f32, tag="zs")
                    nc.vector.tensor_tensor(out=zs[:], in0=z[:],
                                            in1=zm[:].to_broadcast([GPC, C]),
                                            op=ALU.subtract)
                    ez = sp.tile([GPC, C], f32, tag="ez")
                    nc.scalar.activation(ez[:], zs[:], AF.Exp)
                    es = sp.tile([GPC, 1], f32, tag="es")
                    nc.vector.reduce_sum(es[:], ez[:], axis=AX.X)
                    les = sp.tile([GPC, 1], f32, tag="les")
                    nc.scalar.activation(les[:], es[:], AF.Ln)
                    res = sp.tile([GPC, C], f32, tag="res")
                    nc.vector.tensor_tensor(out=res[:], in0=zs[:],
                                            in1=les[:].to_broadcast([GPC, C]),
                                            op=ALU.subtract)
                    nc.sync.dma_start(out_d[:], res[:])

    nc.finalize()
    return nc


# ----------------------------------------------------------------------------
# PJRT runner (cached jit, minimal per-call work)
# ----------------------------------------------------------------------------

def make_runner(nc):
    import jax
    import numpy as _np
    from jax.sharding import Mesh, PartitionSpec
    from jax.experimental.shard_map import shard_map
    import concourse.mybir as mybir
    from concourse import bass2jax as b2j

    b2j.install_neuronx_cc_hook()
    partition_name = nc.partition_id_tensor.name if nc.partition_id_tensor else None
    in_names, out_names, out_avals = [], [], []
    for alloc in nc.m.functions[0].allocations:
        if not isinstance(alloc, mybir.MemoryLocationSet):
            continue
        name = alloc.memorylocations[0].name
        if alloc.kind == "ExternalInput":
            if name != partition_name:
                in_names.append(name)
        elif alloc.kind == "ExternalOutput":
            out_names.append(name)
            shape = tuple(alloc.tensor_shape)
            out_avals.append(jax.core.ShapedArray(shape, mybir.dt.np(alloc.dtype)))
    n_params = len(in_names)
    n_outs = len(out_avals)
    in_names_all = in_names + out_names + \
        ([partition_name] if partition_name else [])
    donate = tuple(range(n_params, n_params + n_outs))

    def _body(*args):
        operands = list(args)
        if partition_name is not None:
            operands.append(b2j.partition_id_tensor())
        outs = b2j._bass_exec_p.bind(
            *operands, out_avals=tuple(out_avals), in_names=tuple(in_names_all),
            out_names=tuple(out_names), lowering_input_output_aliases=(),
            sim_require_finite=True, sim_require_nnan=True, nc=nc)
        return tuple(outs)

    devices = jax.devices()[:NCORES]
    mesh = Mesh(_np.asarray(devices), ("core",))
    in_specs = (PartitionSpec("core"),) * (n_params + n_outs)
    out_specs = (PartitionSpec("core"),) * len(out_names)
    sharded = jax.jit(shard_map(_body, mesh=mesh, in_specs=in_specs,
                                out_specs=out_specs, check_rep=False),
                      donate_argnums=donate, keep_unused=True)

    def run(concat_ins):
        # concat_ins: dict name -> np array concatenated over cores on axis 0
        args = [concat_ins[nm] for nm in in_names]
        zeros = [_np.zeros((NCORES * a.shape[0], *a.shape[1:]), a.dtype)
                 for a in out_avals]
        outs = sharded(*args, *zeros)
        return {nm: _np.asarray(o) for nm, o in zip(out_names, outs)}

    return run, in_names, out_names


# ----------------------------------------------------------------------------
# entry point
# ----------------------------------------------------------------------------

_trace = {"on": False, "res": None}
_cache = {}


def _graph_key(src, dst, batch):
    h = hashlib.blake2b(digest_size=16)
    h.update(np.ascontiguousarray(src).tobytes())
    h.update(np.ascontiguousarray(dst).tobytes())
    h.update(np.ascontiguousarray(batch).tobytes())
    return h.hexdigest()


def _get_state(src, dst, batch):
    key = (_graph_key(src, dst, batch), _trace.get("stage", 5))
    st = _cache.get(key)
    if st is None:
        meta = build_meta(src, dst, batch)
        nc = build_program(meta, stage=_trace.get("stage", 5))
        run, in_names, out_names = make_runner(nc)
        st = dict(meta=meta, nc=nc, run=run,
                  in_names=in_names, out_names=out_names,
                  xp=np.zeros((meta["TBL"], D_IN), BF16))
        _cache.clear()
        _cache[key] = st
    return st


def kernel(**inputs):
    x = np.asarray(inputs["x"], np.float32)
    src = np.asarray(inputs["src"])
    dst = np.asarray(inputs["dst"])
    batch = np.asarray(inputs["batch"])

    st = _get_state(src, dst, batch)
    meta = st["meta"]
    S_pad, TBL = meta["S_pad"], meta["TBL"]

    xp = st["xp"]
    xp[meta["map_row"]] = x.astype(BF16)
    xT = np.ascontiguousarray(
        xp.reshape(NCORES, S_pad, D_IN).transpose(0, 2, 1))  # [NC, D_IN, S_pad]
    W_emb_bf = np.asarray(inputs["W_emb"], np.float32).astype(BF16)
    xT_in = np.concatenate(
        [xT, np.broadcast_to(W_emb_bf, (NCORES, D_IN, H))], axis=2)
    xT_in = np.ascontiguousarray(xT_in).reshape(NCORES * D_IN, S_pad + H)

    wp = pack_weights(inputs)
    wpack = np.ascontiguousarray(np.broadcast_to(wp, (NCORES, 1, WPACK))
                                 ).reshape(NCORES, WPACK)

    concat_ins = {"xT_in": xT_in, "wpack": wpack}
    outs = st["run"](concat_ins)
    _trace["nc"] = st["nc"]
    _trace["in_maps"] = [
        dict(xT_in=xT_in[k * D_IN:(k + 1) * D_IN], wpack=wpack[k:k + 1])
        for k in range(NCORES)]
    out = outs["out"].reshape(NCORES, GPC, C).reshape(G, C)
    return out.astype(np.float32)
